# revision 6
# baseline (speedup 1.0000x reference)
"""DCT heat-blur kernel for Trainium2 (8 NeuronCores, Bass/Tile).

Math: reference computes, per image X (one (batch, channel) slice):
    coefs = D X D^T;  coefs *= E;  out = D^T coefs' D
with E[h,w] = e_h e_w rank-1 (e_k = exp(-(pi k/N)^2 sigma^2/2)), so
    out = W X W,   W = D^T diag(e) D   (symmetric).

Spectral truncation: e_k decays like exp(-(pi k/N)^2 sigma^2/2); rows of D
with e_k below ~1e-3 are dropped.  With Ge = diag(sqrt(e)) D[:K]  (K x N),
W = Ge^T Ge, and the device evaluates the chain
    T1 = X^T Ge^T ; T2 = T1^T Ge^T (= Ge X Ge^T) ; T3 = T2^T Ge ;
    out = T3^T Ge  (= Ge^T Ge X Ge^T Ge = W X W)
as 9 matmuls streaming 6K+768 columns, vs 2048 for the dense 2-GEMM path.
Batches where K would exceed 128 (small sigma) use the dense per-batch-W
2-GEMM path instead.

Sharding/layout: batches are sorted by fwd_step and dealt into 16 blocks of
8; core i takes the i-th batch of each block, so all 8 cores run ONE
program whose per-slot K is the block max.  The program is compiled (and
cached) per slot-plan, derived from the runtime fwd_steps.

Heavily-blurred slots (min step >= FP8_MIN_STEP) ship x as fp8e4m3; the
SWDGE (gpsimd) DMA casts to fp16 on the fly, halving their load traffic.
Output is stored fp16 (tolerance 2e-2, fp16 error ~1e-3).
"""

import os
import numpy as np
import ml_dtypes

BATCH = 128
CHANNELS = 3
N = 256
N_CORES = 8
NSLOT = 16                    # batch slots per core
IM = CHANNELS * 2 * N         # free-dim elems per slot tile [128, 3, 2, 256]

USE_FP8 = os.environ.get("BASS_DCT_FP8", "1") == "1"
FP8_MIN_STEP = int(os.environ.get("BASS_DCT_FP8_MIN_STEP", "6"))
KCAP = 128
K_COEF = float(os.environ.get("BASS_DCT_K_COEF", "604"))  # k_cut = K_COEF/(s+1)

LAST_EXEC_TIME_NS = None
_NC_CACHE = {}
_CONST_CACHE = {}


def _k_of_step(s):
    k = int(np.ceil(K_COEF / (s + 1.0) / 16.0) * 16)
    return max(32, k)


def _slot_plan(steps):
    """Sort batches by step, deal into 16 blocks of 8 (one batch per core).

    Returns (order, slots) where slots[j] = (mode, K, is_fp8) for block j,
    identical across cores (K = block max = K of the smallest step in block).
    """
    order = np.argsort(steps, kind="stable")
    slots = []
    for j in range(NSLOT):
        smin = int(steps[order[8 * j]])
        k = _k_of_step(smin)
        if k > KCAP:
            slots.append(("W", 0, False))
        else:
            slots.append(("G", k, USE_FP8 and smin >= FP8_MIN_STEP))
    return order, slots


def _install_ntff_hook():
    """Wire antenv.axon_hooks (missing in this image) so trace=True works."""
    import sys
    import types

    if "antenv.axon_hooks" in sys.modules:
        return
    try:
        import trn_agent_boot.trn_boot as tb

        hook = tb._ntff_profile_via_ctypes("/opt/axon/libaxon_pjrt.so")
    except Exception:
        hook = None
    m = types.ModuleType("antenv.axon_hooks")
    m.get_axon_ntff_profile_hook = lambda: hook
    m.set_axon_ntff_profile_hook = lambda h: None
    sys.modules["antenv.axon_hooks"] = m


def _layout(slots):
    """Shared (cross-core) layout derived from the slot plan.

    proc: processing order (lightest slots first). xpos: slot -> index in its
    x dram tensor. coff: slot -> const-column info. bounds: const chunk
    boundaries (4 chunks -> independent tiles so early slots don't wait on
    the whole constant transfer).
    """
    proc = list(range(NSLOT - 1, -1, -1))
    xpos = {}
    n16 = n8 = 0
    coff = {}
    ccols = 0
    for j in proc:
        mode, K, is8 = slots[j]
        if is8:
            xpos[j] = n8
            n8 += 1
        else:
            xpos[j] = n16
            n16 += 1
        if mode == "W":
            coff[j] = ("W", ccols, 2 * N)
            ccols += 2 * N
        else:
            coff[j] = ("G", ccols, 2 * K + N)
            ccols += 2 * K + N
    # chunk const at slot boundaries into 4 groups (proc order)
    bounds = [0]
    per = (NSLOT + 3) // 4
    acc = 0
    for q in range(4):
        js = proc[q * per : (q + 1) * per]
        acc += sum(coff[j][2] for j in js)
        bounds.append(acc)
    return proc, xpos, n16, n8, coff, ccols, bounds


def _build_nc(key):
    import concourse.bacc as bacc
    import concourse.tile as tile
    import concourse.mybir as mybir

    f32 = mybir.dt.float32
    f16 = mybir.dt.float16
    f8 = mybir.dt.float8e4

    slots = list(key)
    proc, xpos, n16, n8, coff, ccols, bounds = _layout(slots)

    nc = bacc.Bacc("TRN2", target_bir_lowering=False, debug=False)
    x16_d = (
        nc.dram_tensor("x16", [n16, 128, IM], f16, kind="ExternalInput").ap()
        if n16
        else None
    )
    x8_d = (
        nc.dram_tensor("x8", [n8, 128, IM], f8, kind="ExternalInput").ap()
        if n8
        else None
    )
    gw_d = nc.dram_tensor("gw", [128, ccols], f16, kind="ExternalInput").ap()
    o_d = nc.dram_tensor("o", [NSLOT, 128, IM], f16, kind="ExternalOutput").ap()

    with tile.TileContext(nc) as tc:
        with (
            tc.tile_pool(name="const", bufs=1) as cpool,
            tc.tile_pool(name="xpool", bufs=NSLOT + 1) as xpool,
            tc.tile_pool(name="t1p", bufs=4) as t1pool,
            tc.tile_pool(name="t2p", bufs=4) as t2pool,
            tc.tile_pool(name="t3p", bufs=4) as t3pool,
            tc.tile_pool(name="opool", bufs=4) as opool,
            tc.tile_pool(name="ps1", bufs=2, space="PSUM") as ps1,
            tc.tile_pool(name="ps2", bufs=2, space="PSUM") as ps2,
            tc.tile_pool(name="ps3", bufs=2, space="PSUM") as ps3,
            tc.tile_pool(name="ps4", bufs=2, space="PSUM") as ps4,
        ):
            # constants in 4 independent chunk tiles (proc-ordered cols)
            ctiles = []
            for q in range(4):
                lo, hi = bounds[q], bounds[q + 1]
                if hi == lo:
                    ctiles.append(None)
                    continue
                ct = cpool.tile([128, hi - lo], f16, name=f"gw{q}")
                if q == 0:
                    with tc.high_priority():
                        nc.sync.dma_start(ct[:], gw_d[:, lo:hi])
                else:
                    (nc.scalar if q % 2 else nc.sync).dma_start(
                        ct[:], gw_d[:, lo:hi]
                    )
                ctiles.append(ct)

            def cslice(j):
                base = coff[j][1]
                q = 0
                while bounds[q + 1] <= base:
                    q += 1
                return ctiles[q], base - bounds[q]

            # prefetch all x tiles (fp8 slots via SWDGE with cast-to-fp16)
            xts = {}
            for j in proc:
                mode, K, is8 = slots[j]
                xt = xpool.tile([128, CHANNELS, 2, N], f16, tag="xt")
                src = (x8_d if is8 else x16_d)[xpos[j]].rearrange(
                    "p (c a w) -> p c a w", c=CHANNELS, a=2
                )
                (nc.gpsimd if is8 else nc.sync).dma_start(xt[:], src)
                xts[j] = xt

            cp = [nc.vector.tensor_copy, nc.scalar.copy]

            def copy(i, dst, src):
                if i % 2 == 0:
                    nc.vector.tensor_copy(out=dst, in_=src)
                else:
                    nc.scalar.copy(dst, src)

            for idx, j in enumerate(proc):
                mode, K, is8 = slots[j]
                xt = xts.pop(j)
                ct, base = cslice(j)
                ot = opool.tile([128, CHANNELS, 2, N], f16, tag="ot")
                for c in range(CHANNELS):
                    im = idx * CHANNELS + c
                    if mode == "W":
                        t1_ps = ps1.tile([128, 2, N], f32, tag="t1")
                        for mb in range(2):
                            for a in range(2):
                                nc.tensor.matmul(
                                    t1_ps[:, mb, :],
                                    lhsT=xt[:, c, a, mb * 128 : (mb + 1) * 128],
                                    rhs=ct[:, base + a * N : base + (a + 1) * N],
                                    start=(a == 0),
                                    stop=(a == 1),
                                )
                        t1_sb = t1pool.tile([128, 2, N], f16, tag="t1sb")
                        copy(im, t1_sb[:], t1_ps[:])
                        o_ps = ps4.tile([128, 2, N], f32, tag="ops")
                        for mb in range(2):
                            for a in range(2):
                                nc.tensor.matmul(
                                    o_ps[:, mb, :],
                                    lhsT=t1_sb[:, a, mb * 128 : (mb + 1) * 128],
                                    rhs=ct[:, base + a * N : base + (a + 1) * N],
                                    start=(a == 0),
                                    stop=(a == 1),
                                )
                        copy(im + 1, ot[:, c], o_ps[:])
                    else:
                        gT = base          # Ge^T block: [p, a*K + n]
                        ge = base + 2 * K  # Ge block: rows :K, cols [ge, ge+N)
                        t1_ps = ps1.tile([128, 2, K], f32, tag="t1")
                        for mb in range(2):
                            for a in range(2):
                                nc.tensor.matmul(
                                    t1_ps[:, mb, :],
                                    lhsT=xt[:, c, a, mb * 128 : (mb + 1) * 128],
                                    rhs=ct[:, gT + a * K : gT + (a + 1) * K],
                                    start=(a == 0),
                                    stop=(a == 1),
                                )
                        t1_sb = t1pool.tile([128, 2, K], f16, tag="t1sb")
                        copy(im, t1_sb[:], t1_ps[:])
                        t2_ps = ps2.tile([K, K], f32, tag="t2")
                        for a in range(2):
                            nc.tensor.matmul(
                                t2_ps[:, :],
                                lhsT=t1_sb[:, a, :],
                                rhs=ct[:, gT + a * K : gT + (a + 1) * K],
                                start=(a == 0),
                                stop=(a == 1),
                            )
                        t2_sb = t2pool.tile([K, K], f16, tag="t2sb")
                        copy(im + 1, t2_sb[:], t2_ps[:])
                        t3_ps = ps3.tile([K, N], f32, tag="t3")
                        nc.tensor.matmul(
                            t3_ps[:, :],
                            lhsT=t2_sb[:, :],
                            rhs=ct[:K, ge : ge + N],
                            start=True,
                            stop=True,
                        )
                        t3_sb = t3pool.tile([K, N], f16, tag="t3sb")
                        copy(im, t3_sb[:], t3_ps[:])
                        o_ps = ps4.tile([128, 2, N], f32, tag="ops")
                        for mb in range(2):
                            nc.tensor.matmul(
                                o_ps[:, mb, :],
                                lhsT=t3_sb[:, mb * 128 : (mb + 1) * 128],
                                rhs=ct[:K, ge : ge + N],
                                start=True,
                                stop=True,
                            )
                        copy(im + 1, ot[:, c], o_ps[:])
                    # late slots: store per channel from the (idle) sync ring
                    if idx >= NSLOT - 4:
                        nc.sync.dma_start(
                            o_d[j].rearrange(
                                "p (c a w) -> p c a w", c=CHANNELS, a=2
                            )[:, c],
                            ot[:, c],
                        )
                if idx < NSLOT - 4:
                    nc.scalar.dma_start(
                        o_d[j].rearrange("p (c a w) -> p c a w", c=CHANNELS, a=2),
                        ot[:],
                    )

    nc.compile()
    return nc


def _get_nc(key):
    if key not in _NC_CACHE:
        _NC_CACHE[key] = _build_nc(key)
    return _NC_CACHE[key]


def _dct_consts():
    if "D" not in _CONST_CACHE:
        n = np.arange(N, dtype=np.float64)
        D = np.sqrt(2.0 / N) * np.cos(np.pi * (n[None, :] + 0.5) * n[:, None] / N)
        D[0] *= 1.0 / np.sqrt(2.0)
        _CONST_CACHE["D"] = D
        _CONST_CACHE["freqs"] = np.pi * n / N
    return _CONST_CACHE["D"], _CONST_CACHE["freqs"]


def _e_of(sigma):
    D, freqs = _dct_consts()
    t = float(sigma) ** 2 / 2.0
    return np.exp(-(freqs**2) * t)


def _ge_block(sigma, K):
    """[128, 2K+N] fp16 for the G-chain.

    The reference operator is out = M X M^T with M = D diag(e) D (the
    "inverse" einsum applies D again, not D^T).  Truncating the contraction
    at K:  M ~= F^T G with G = diag(sqrt(e)) D[:K]  and  F = diag(sqrt(e))
    D^T[:K], so  out = F^T (G X G^T) F.  The device chain
        T1 = X^T G^T; T2 = T1^T G^T (= G X G^T); T3 = T2^T F; out = T3^T F
    needs rhs blocks G^T (cols [0, 2K), partition = space) and F
    (cols [2K, 2K+N), partitions = spectral :K).
    """
    D, _ = _dct_consts()
    e = _e_of(sigma)
    se = np.sqrt(e[:K])
    Ge = se[:, None] * D[:K]  # [K, N]
    F = se[:, None] * D[:, :K].T  # [K, N] = diag(sqrt(e)) D^T[:K]
    blk = np.zeros((128, 2 * K + N), dtype=np.float16)
    # geT[p, a*K + n] = Ge[n, a*128 + p]
    blk[:, : 2 * K] = (
        Ge.T.reshape(2, 128, K).transpose(1, 0, 2).reshape(128, 2 * K)
    )
    blk[:K, 2 * K :] = F.astype(np.float16)
    return blk


def _w_block(sigma):
    """[128, 2N] fp16: dense W = (D diag(e) D)^T in [p, a*N + h] layout."""
    D, _ = _dct_consts()
    e = _e_of(sigma)
    W = (D @ (e[:, None] * D)).T  # [N, N] = M^T
    return np.ascontiguousarray(
        W.reshape(2, 128, N).transpose(1, 0, 2).reshape(128, 2 * N)
    ).astype(np.float16)


def kernel(x, blur_sigmas, fwd_steps):
    global LAST_EXEC_TIME_NS
    from concourse import bass_utils

    x = np.ascontiguousarray(np.asarray(x), dtype=np.float32)
    assert x.shape == (BATCH, CHANNELS, N, N), x.shape
    sig = np.asarray(blur_sigmas, dtype=np.float64)
    steps = np.asarray(fwd_steps).astype(np.int64)

    order, slots = _slot_plan(steps)
    key = tuple(slots)
    proc, xpos, n16, n8, coff, ccols, bounds = _layout(key)
    nc = _get_nc(key)

    # per-step block caches (few unique steps)
    geb = {}
    wb = {}

    in_maps = []
    for i in range(N_CORES):
        x16 = np.empty((n16, 128, IM), dtype=np.float16) if n16 else None
        x8 = (
            np.empty((n8, 128, IM), dtype=ml_dtypes.float8_e4m3) if n8 else None
        )
        gw = np.empty((128, ccols), dtype=np.float16)
        for j in proc:
            mode, K, is8 = slots[j]
            b = int(order[8 * j + i])
            s = int(steps[b])
            img = (
                x[b]
                .reshape(CHANNELS, 2, 128, N)
                .transpose(2, 0, 1, 3)
                .reshape(128, IM)
            )
            if is8:
                x8[xpos[j]] = img.astype(ml_dtypes.float8_e4m3)
            else:
                x16[xpos[j]] = img.astype(np.float16)
            base = coff[j][1]
            if mode == "W":
                if s not in wb:
                    wb[s] = _w_block(sig[s])
                gw[:, base : base + 2 * N] = wb[s]
            else:
                if (s, K) not in geb:
                    geb[(s, K)] = _ge_block(sig[s], K)
                gw[:, base : base + 2 * K + N] = geb[(s, K)]
        m = {"gw": gw}
        if n16:
            m["x16"] = x16
        if n8:
            m["x8"] = x8
        in_maps.append(m)

    trace = os.environ.get("BASS_DCT_TRACE", "0") == "1"
    kwargs = {}
    if trace:
        _install_ntff_hook()
        kwargs["trace"] = True
        tmpdir = os.environ.get("BASS_DCT_TRACE_DIR")
        if tmpdir:
            kwargs["tmpdir"] = tmpdir
    res = None
    for attempt in range(3):
        try:
            res = bass_utils.run_bass_kernel_spmd(
                nc, in_maps, core_ids=list(range(N_CORES)), **kwargs
            )
            break
        except Exception:
            # transient NRT_EXEC_UNIT_UNRECOVERABLE has been observed on the
            # first execution of a freshly loaded NEFF; a retry succeeds
            if attempt == 2:
                raise
            import time as _time

            _time.sleep(2.0)
            kwargs.pop("trace", None)
            kwargs.pop("tmpdir", None)
    LAST_EXEC_TIME_NS = res.exec_time_ns

    out = np.empty((BATCH, CHANNELS, N, N), dtype=np.float32)
    for i in range(N_CORES):
        oc = res.results[i]["o"]  # [NSLOT, 128, IM] fp16
        for j in range(NSLOT):
            b = int(order[8 * j + i])
            out[b] = (
                oc[j]
                .reshape(128, CHANNELS, 2, N)
                .transpose(1, 2, 0, 3)
                .reshape(CHANNELS, N, N)
                .astype(np.float32)
            )
    return out
